# revision 28
# baseline (speedup 1.0000x reference)
"""Trainium2 Bass kernel for nn_GAT_T (2x GATConv + dense self-attention hybrid).

Sharding across 8 NeuronCores: core c owns nodes [1024c, 1024(c+1)).
 - GATConv layers: dst-node sharded, dense masked-softmax formulation.
   The adjacency mask is built on-device chunk-by-chunk with gpsimd
   local_scatter from host-packed int16 index tables (pure index reformat).
 - Dense NxN attention: query-row sharded; K/V computed per-core on the
   owned node slice and AllGathered during the GAT1 phase (off the
   critical path), then streamed from the gathered DRAM buffers.
 - AllGathers (1MB/core each, bf16): k, v during GAT1; l1 and h2-own
   between the GAT layers, in flight during the attention phase.
Heavy matmuls in bf16 with fp32 PSUM accumulation. Softmax computed without
max-subtraction (logits are O(+-6); mathematically identical).
"""

import numpy as np

NA, NB = 4096, 4096
N = NA + NB
IN, H = 256, 512
N_CORES = 8
NO = N // N_CORES      # 1024 nodes per core
KT = N // 128          # 64 src chunks
DH = NO // 512         # 2 dst halves
NEG_SLOPE = 0.2

TRACE = False
DEBUG = False
LAST_EXEC_NS = None
_LAST_RES = None
_CACHE = {}


def _install_trace_hook():
    import sys, types
    if "antenv.axon_hooks" in sys.modules:
        return
    try:
        mod = types.ModuleType("antenv.axon_hooks")
        mod._hook = None
        mod.set_axon_ntff_profile_hook = lambda h: setattr(mod, "_hook", h)
        mod.get_axon_ntff_profile_hook = lambda: mod._hook
        sys.modules["antenv.axon_hooks"] = mod
        from trn_agent_boot.trn_boot import _ntff_profile_via_ctypes
        mod.set_axon_ntff_profile_hook(
            _ntff_profile_via_ctypes("/opt/axon/libaxon_pjrt.so"))
    except Exception:
        pass


def _build(W):
    import concourse.bacc as bacc
    import concourse.mybir as mybir
    import concourse.tile as tile

    f32 = mybir.dt.float32
    bf16 = mybir.dt.bfloat16
    i16 = mybir.dt.int16
    AF = mybir.ActivationFunctionType

    nc = bacc.Bacc("TRN2", target_bir_lowering=False, debug=False,
                   num_devices=N_CORES)

    def inp(name, shape, dt=f32):
        return nc.dram_tensor(name, shape, dt, kind="ExternalInput")

    xT = inp("xT", [IN, N])
    xoT = inp("xoT", [IN, NO])
    WinA_T = inp("WinA_T", [IN, H]); WinB_T = inp("WinB_T", [IN, H])
    Win2A_T = inp("Win2A_T", [IN, H]); Win2B_T = inp("Win2B_T", [IN, H])
    win_o = inp("win_o", [IN, H]); win2_o = inp("win2_o", [IN, H])
    binA = inp("binA", [H, 1]); binB = inp("binB", [H, 1])
    bin2A = inp("bin2A", [H, 1]); bin2B = inp("bin2B", [H, 1])
    bin_o = inp("bin_o", [H, 1]); bin2_o = inp("bin2_o", [H, 1])
    Wg1 = inp("Wg1", [H, H]); Wg1_T = inp("Wg1_T", [H, H]); A1 = inp("A1", [H, 2])
    Wg2 = inp("Wg2", [H, H]); Wg2_T = inp("Wg2_T", [H, H]); A2 = inp("A2", [H, 2])
    bg1 = inp("bg1", [H, 1]); bg2 = inp("bg2", [H, 1])
    WqT = inp("WqT", [H, H]); WkT = inp("WkT", [H, H]); WvT = inp("WvT", [H, H])
    bq = inp("bq", [H, 1]); bk = inp("bk", [H, 1]); bv = inp("bv", [H, 1])
    WoT = inp("WoT", [H, H]); bo = inp("bo", [H, 1])
    sc_idx = inp("sc_idx", [128, KT * W], i16)
    sc_cnt = inp("sc_cnt", [128, KT * W], bf16)

    out_l = nc.dram_tensor("out_l", [4, 128, NO], f32, kind="ExternalOutput")
    out_g = nc.dram_tensor("out_g", [4, 128, NO], f32, kind="ExternalOutput")
    if DEBUG:
        dbg_h1 = nc.dram_tensor("dbg_h1", [KT, 128, 516], mybir.dt.bfloat16, kind="ExternalOutput")
        dbg_ssrc = nc.dram_tensor("dbg_ssrc", [128, KT], f32, kind="ExternalOutput")
        dbg_sdb = nc.dram_tensor("dbg_sdb", [128, NO], f32, kind="ExternalOutput")
        dbg_l1 = nc.dram_tensor("dbg_l1", [4, 128, NO], mybir.dt.bfloat16, kind="ExternalOutput")
        dbg_l1g = nc.dram_tensor("dbg_l1g", [N_CORES, 4, 128, NO], mybir.dt.bfloat16, kind="ExternalOutput")
        dbg_mask = nc.dram_tensor("dbg_mask", [KT, 128, 512], mybir.dt.bfloat16, kind="ExternalOutput")

    RG = [list(range(N_CORES))]
    SCL = 1.0 / float(np.sqrt(H))

    with tile.TileContext(nc) as tc:
        with (
            tc.tile_pool(name="wp", bufs=1) as wp,
            tc.tile_pool(name="apool", bufs=1) as ap,
            tc.tile_pool(name="own", bufs=2) as op_,
            tc.tile_pool(name="sp", bufs=3) as sp,
            tc.tile_pool(name="rp", bufs=2) as rp,
            tc.tile_pool(name="lp", bufs=3) as lp,
            tc.tile_pool(name="pp", bufs=1, space="PSUM") as pp,
            tc.tile_pool(name="ppmm", bufs=2, space="PSUM") as ppmm,
            tc.tile_pool(name="dram", bufs=1, space="DRAM") as dp,
        ):
            # ---------- load weights (cast to bf16) ----------
            def w16(dram, rows, cols, tag):
                t = wp.tile([128, rows // 128, cols], bf16, tag=tag)
                nc.gpsimd.dma_start(
                    t[:], dram[:].rearrange("(a p) c -> p a c", p=128))
                return t

            def bias32(dram, tag):
                t = wp.tile([128, H // 128], f32, tag=tag)
                nc.gpsimd.dma_start(
                    t[:], dram[:].rearrange("(a p) one -> p (a one)", p=128))
                return t

            # stage-0 weights first (POOL queue is in-order)
            wA = w16(WinA_T, IN, H, "wA"); wB = w16(WinB_T, IN, H, "wB")
            w2A = w16(Win2A_T, IN, H, "w2A"); w2B = w16(Win2B_T, IN, H, "w2B")
            wo_ = w16(win_o, IN, H, "wo_"); w2o_ = w16(win2_o, IN, H, "w2o_")
            bAf = bias32(binA, "bAf"); bBf = bias32(binB, "bBf")
            b2Af = bias32(bin2A, "b2Af"); b2Bf = bias32(bin2B, "b2Bf")
            bof_ = bias32(bin_o, "bof_"); b2of_ = bias32(bin2_o, "b2of_")
            bqf = bias32(bq, "bqf")
            qT = w16(WqT, H, H, "qT")
            # weights needed by GAT1 prep
            g1 = w16(Wg1, H, H, "g1"); g1T = w16(Wg1_T, H, H, "g1T")
            a1 = w16(A1, H, 2, "a1")
            bg1f = bias32(bg1, "bg1f")
            # remaining weights (kv, GAT2, output) load during stage 0
            kT = w16(WkT, H, H, "kT"); vT = w16(WvT, H, H, "vT")
            oT = w16(WoT, H, H, "oT")
            g2 = w16(Wg2, H, H, "g2"); g2T = w16(Wg2_T, H, H, "g2T")
            a2 = w16(A2, H, 2, "a2")
            bkf = bias32(bk, "bkf"); bg2f = bias32(bg2, "bg2f")
            bof2 = bias32(bo, "bof2")
            sci = wp.tile([128, KT * W], i16, tag="sci")
            nc.scalar.dma_start(sci[:], sc_idx[:])
            scc = wp.tile([128, KT * W], bf16, tag="scc")
            nc.scalar.dma_start(scc[:], sc_cnt[:])
            ones_r = wp.tile([1, 128], f32, tag="ones_r")
            nc.vector.memset(ones_r[:], 1.0)
            ones_c = wp.tile([128, 1], f32, tag="ones_c")
            nc.vector.memset(ones_c[:], 1.0)
            # bv as a broadcast row [128, 512] f32
            bvrow = wp.tile([1, H], f32, tag="bvrow")
            nc.gpsimd.dma_start(bvrow[:], bv[:].rearrange("f o -> o f"))
            bvb = wp.tile([128, H], f32, tag="bvb")
            pb = ppmm.tile([128, H], f32, tag="mm")
            nc.tensor.matmul(pb[:], lhsT=ones_r[:], rhs=bvrow[:], start=True, stop=True)
            nc.vector.tensor_copy(bvb[:], pb[:])

            # ---------- internal DRAM ----------
            l0_dram = dp.tile([4, 128, N], bf16, tag="l0")
            h_dram1 = dp.tile([KT, 128, 512], bf16, tag="h1")
            ko_b = dp.tile([4, 128, NO], bf16, tag="kob")
            vo_b = dp.tile([8, 128, 512], bf16, tag="vob")
            kg = dp.tile([N_CORES, 4, 128, NO], bf16, tag="kg",
                         addr_space="Shared")
            vg = dp.tile([N_CORES, 8, 128, 512], bf16, tag="vg",
                         addr_space="Shared")
            l1own_b = dp.tile([4, 128, NO], bf16, tag="l1ob")
            h2o_b = dp.tile([8, 128, 512], bf16, tag="h2ob")
            h2g = dp.tile([N_CORES, 8, 128, 512], bf16, tag="h2g",
                          addr_space="Shared")
            S_dram = dp.tile([KT, 128, NO], bf16, tag="Sd")
            ssrc_stage = dp.tile([1, N], f32, tag="sstage")
            l1g = dp.tile([N_CORES, 4, 128, NO], bf16, tag="l1g", addr_space="Shared")

            # ---------- stage 0: input linears ----------
            # l0' and g0' full (feature-major, bf16) -> DRAM
            for n16 in range(16):
                xqf = lp.tile([128, 2, 512], f32, tag="xqf", bufs=2)
                nc.scalar.dma_start(
                    xqf[:], xT[:, 512 * n16:512 * (n16 + 1)]
                    .rearrange("(a p) c -> p a c", p=128))
                xq = lp.tile([128, 2, 512], bf16, tag="xq", bufs=2)
                nc.vector.tensor_copy(xq[:], xqf[:])
                wl = wA if n16 < 8 else wB
                wg = w2A if n16 < 8 else w2B
                bl = bAf if n16 < 8 else bBf
                bg_ = b2Af if n16 < 8 else b2Bf
                for mp in range(2):
                    st2 = sp.tile([128, 2, 512], bf16, tag="stg2")
                    for mi in range(2):
                        m = 2 * mp + mi
                        ps = ppmm.tile([128, 512], f32, tag="mm")
                        for k2 in range(2):
                            nc.tensor.matmul(
                                ps[:], lhsT=wl[:, k2, 128 * m:128 * (m + 1)],
                                rhs=xq[:, k2, :],
                                start=(k2 == 0), stop=(k2 == 1))
                        nc.vector.tensor_scalar_add(
                            st2[:, mi, :], ps[:], bl[:, m:m + 1])
                    nc.sync.dma_start(
                        l0_dram[2 * mp:2 * mp + 2, :, 512 * n16:512 * (n16 + 1)]
                        .rearrange("a p c -> p a c"), st2[:])


            # own slices: l0o, g0o (SBUF resident) from xoT
            l0o = op_.tile([128, 4, NO], bf16, tag="own")
            g0o = op_.tile([128, 4, NO], bf16, tag="own")
            for n2 in range(2):
                xqf = lp.tile([128, 2, 512], f32, tag="xqf", bufs=2)
                nc.scalar.dma_start(
                    xqf[:], xoT[:, 512 * n2:512 * (n2 + 1)]
                    .rearrange("(a p) c -> p a c", p=128))
                xq = lp.tile([128, 2, 512], bf16, tag="xq", bufs=2)
                nc.vector.tensor_copy(xq[:], xqf[:])
                for m in range(4):
                    ps = ppmm.tile([128, 512], f32, tag="mm")
                    for k2 in range(2):
                        nc.tensor.matmul(
                            ps[:], lhsT=wo_[:, k2, 128 * m:128 * (m + 1)],
                            rhs=xq[:, k2, :], start=(k2 == 0), stop=(k2 == 1))
                    nc.vector.tensor_scalar_add(
                        l0o[:, m, 512 * n2:512 * (n2 + 1)], ps[:], bof_[:, m:m + 1])
                for m in range(4):
                    ps = ppmm.tile([128, 512], f32, tag="mm")
                    for k2 in range(2):
                        nc.tensor.matmul(
                            ps[:], lhsT=w2o_[:, k2, 128 * m:128 * (m + 1)],
                            rhs=xq[:, k2, :], start=(k2 == 0), stop=(k2 == 1))
                    nc.vector.tensor_scalar_add(
                        g0o[:, m, 512 * n2:512 * (n2 + 1)], ps[:], b2of_[:, m:m + 1])

            # q' own (feature-major bf16, bias added)
            q16 = ap.tile([128, 4, NO], bf16, tag="q16")
            for n2 in range(2):
                for m in range(4):
                    ps = ppmm.tile([128, 512], f32, tag="mm")
                    for k2 in range(4):
                        nc.tensor.matmul(
                            ps[:], lhsT=qT[:, k2, 128 * m:128 * (m + 1)],
                            rhs=g0o[:, k2, 512 * n2:512 * (n2 + 1)],
                            start=(k2 == 0), stop=(k2 == 3))
                    nc.vector.tensor_scalar_add(
                        q16[:, m, 512 * n2:512 * (n2 + 1)], ps[:], bqf[:, m:m + 1])

            # ---------- own-slice K/V + AllGather (in flight during GAT1) ----
            for n2 in range(2):
                st2 = sp.tile([128, 2, 512], bf16, tag="stg2")
                for mp in range(2):
                    for mi in range(2):
                        m = 2 * mp + mi
                        ps = ppmm.tile([128, 512], f32, tag="mm")
                        for k2 in range(4):
                            nc.tensor.matmul(
                                ps[:], lhsT=kT[:, k2, 128 * m:128 * (m + 1)],
                                rhs=g0o[:, k2, 512 * n2:512 * (n2 + 1)],
                                start=(k2 == 0), stop=(k2 == 3))
                        nc.vector.tensor_scalar_add(
                            st2[:, mi, :], ps[:], bkf[:, m:m + 1])
                    nc.sync.dma_start(
                        ko_b[2 * mp:2 * mp + 2, :, 512 * n2:512 * (n2 + 1)]
                        .rearrange("a p c -> p a c"), st2[:])
                    st2 = sp.tile([128, 2, 512], bf16, tag="stg2")
            for tp in range(4):
                st2 = sp.tile([128, 2, 512], bf16, tag="stg2")
                for ti in range(2):
                    t = 2 * tp + ti
                    ps = ppmm.tile([128, 512], f32, tag="mm")
                    for k2 in range(4):
                        nc.tensor.matmul(
                            ps[:], lhsT=g0o[:, k2, 128 * t:128 * (t + 1)],
                            rhs=vT[:, k2, :], start=(k2 == 0), stop=(k2 == 3))
                    nc.vector.tensor_add(st2[:, ti, :], ps[:], bvb[:])
                nc.sync.dma_start(
                    vo_b[2 * tp:2 * tp + 2, :, :].rearrange("a p c -> p a c"),
                    st2[:])
            nc.gpsimd.collective_compute(
                "AllGather", mybir.AluOpType.bypass,
                replica_groups=RG, ins=[ko_b.opt()], outs=[kg.opt()])
            nc.gpsimd.collective_compute(
                "AllGather", mybir.AluOpType.bypass,
                replica_groups=RG, ins=[vo_b.opt()], outs=[vg.opt()])

            # ---------- mask build (POOL, overlaps PE work) ----------
            for kp in range(KT // 2):
                mf2 = sp.tile([128, 2, NO], bf16, tag="mfull", bufs=2)
                for ki in range(2):
                    off = (2 * kp + ki) * W
                    nc.gpsimd.local_scatter(
                        out_ap=mf2[:, ki, :], data_ap=scc[:, off:off + W],
                        idxs_ap=sci[:, off:off + W],
                        channels=128, num_elems=NO, num_idxs=W)
                nc.sync.dma_start(
                    S_dram[2 * kp:2 * kp + 2, :, :].rearrange("a p c -> p a c"),
                    mf2[:])

            # ---------- helpers ----------
            def compute_h(h_dram, gT_w, lsrc_dram, lsrc_g):
                """h_aug (node-major, 513 cols w/ ones) -> h_dram.
                lsrc either from DRAM [4,128,N] (lsrc_dram) or l1g (lsrc_g)."""
                for tp in range(KT // 2):
                    t0 = 2 * tp
                    if lsrc_dram is not None:
                        lt = lp.tile([128, 4, 256], bf16, tag="hq")
                        nc.scalar.dma_start(
                            lt[:], lsrc_dram[:, :, 128 * t0:128 * (t0 + 2)]
                            .rearrange("a p c -> p a c"))
                    else:
                        c = t0 // 8; dl = t0 % 8
                        lt = lp.tile([128, 4, 256], bf16, tag="hq")
                        nc.scalar.dma_start(
                            lt[:], lsrc_g[c, :, :, 128 * dl:128 * (dl + 2)]
                            .rearrange("a p c -> p a c"))
                    st2 = sp.tile([128, 2, 512], bf16, tag="stg2")
                    for ti in range(2):
                        ps = ppmm.tile([128, 512], f32, tag="mm")
                        for k2 in range(4):
                            nc.tensor.matmul(
                                ps[:], lhsT=lt[:, k2, 128 * ti:128 * (ti + 1)],
                                rhs=gT_w[:, k2, :],
                                start=(k2 == 0), stop=(k2 == 3))
                        nc.vector.tensor_copy(st2[:, ti, :], ps[:])
                    nc.scalar.dma_start(
                        h_dram[t0:t0 + 2, :, 0:512].rearrange("a p c -> p a c"),
                        st2[:])

            def compute_wsd(gw, aw, tag):
                wsd = ap.tile([128, 4, 2], bf16, tag=tag)
                for m in range(4):
                    psw = ppmm.tile([128, 2], f32, tag="mm")
                    for k2 in range(4):
                        nc.tensor.matmul(
                            psw[:], lhsT=gw[:, k2, 128 * m:128 * (m + 1)],
                            rhs=aw[:, k2, :], start=(k2 == 0), stop=(k2 == 3))
                    nc.vector.tensor_copy(wsd[:, m, :], psw[:])
                return wsd

            def compute_ssrc_full(wsd, lsrc_dram, lsrc_g, tag):
                """ssrc for all nodes -> [128, KT] f32 (node 128k+p at [p,k])."""
                sc = ap.tile([128, KT], f32, tag=tag)
                for n16 in range(16):
                    if lsrc_dram is not None:
                        rq = lp.tile([128, 4, 512], bf16, tag="rq")
                        nc.scalar.dma_start(
                            rq[:], lsrc_dram[:, :, 512 * n16:512 * (n16 + 1)]
                            .rearrange("a p c -> p a c"))
                    else:
                        c = n16 // 2; dl = n16 % 2
                        rq = lp.tile([128, 4, 512], bf16, tag="rq")
                        nc.scalar.dma_start(
                            rq[:], lsrc_g[c, :, :, 512 * dl:512 * (dl + 1)]
                            .rearrange("a p c -> p a c"))
                    pss = pp.tile([1, 512], f32, tag="den")
                    for k2 in range(4):
                        nc.tensor.matmul(
                            pss[:], lhsT=wsd[:, k2, 0:1], rhs=rq[:, k2, :],
                            start=(k2 == 0), stop=(k2 == 3))
                    row = rp.tile([1, 512], f32, tag="row")
                    nc.vector.tensor_copy(row[:], pss[:])
                    nc.sync.dma_start(
                        ssrc_stage[:, 512 * n16:512 * (n16 + 1)], row[:])
                nc.sync.dma_start(
                    sc[:], ssrc_stage[0:1, :].rearrange("o (t p) -> p (o t)", p=128))
                return sc

            def compute_sdst_b(wsd, lown, tag):
                """sdst over own nodes, broadcast to [128, NO] f32."""
                sdb = ap.tile([128, NO], f32, tag=tag)
                for n2 in range(2):
                    psd = pp.tile([1, 512], f32, tag="den")
                    for k2 in range(4):
                        nc.tensor.matmul(
                            psd[:], lhsT=wsd[:, k2, 1:2],
                            rhs=lown[:, k2, 512 * n2:512 * (n2 + 1)],
                            start=(k2 == 0), stop=(k2 == 3))
                    row = rp.tile([1, 512], f32, tag="row")
                    nc.vector.tensor_copy(row[:], psd[:])
                    psb = ppmm.tile([128, 512], f32, tag="mm")
                    nc.tensor.matmul(psb[:], lhsT=ones_r[:], rhs=row[:],
                                     start=True, stop=True)
                    nc.vector.tensor_copy(sdb[:, 512 * n2:512 * (n2 + 1)], psb[:])
                return sdb

            def gat_loop(h_dram, ssrc_c, sdb, write_out, dump_mask=False,
                         h_g=None):
                for j in range(DH):
                    aggs = [pp.tile([128, 512], f32, tag=f"agg{m}",
                                    name=f"agg{m}") for m in range(4)]
                    wsum = rp.tile([128, 512], f32, tag="wsum")
                    for kp in range(KT // 2):
                        ht2 = lp.tile([128, 2, 512], bf16, tag="hstream")
                        if h_g is None:
                            nc.sync.dma_start(
                                ht2[:], h_dram[2 * kp:2 * kp + 2, :, 0:512]
                                .rearrange("a p c -> p a c"))
                        else:
                            cr2 = (2 * kp) // 8
                            dl2 = (2 * kp) % 8
                            nc.sync.dma_start(
                                ht2[:], h_g[cr2, dl2:dl2 + 2, :, :]
                                .rearrange("a p c -> p a c"))
                        mk2 = sp.tile([128, 2, 512], bf16, tag="mask")
                        nc.sync.dma_start(
                            mk2[:], S_dram[2 * kp:2 * kp + 2, :,
                                           512 * j:512 * (j + 1)]
                            .rearrange("a p c -> p a c"))
                        wts = []
                        for ki in range(2):
                            k = 2 * kp + ki
                            ht = ht2[:, ki, :]
                            mk = mk2[:, ki, :]
                            et = sp.tile([128, 512], f32, tag="et", bufs=4)
                            nc.scalar.activation(
                                et[:], sdb[:, 512 * j:512 * (j + 1)],
                                AF.Prelu, bias=ssrc_c[:, k:k + 1], scale=1.0,
                                alpha=NEG_SLOPE)
                            pt = sp.tile([128, 512], bf16, tag="pt", bufs=4)
                            nc.scalar.activation(pt[:], et[:], AF.Exp)
                            wt = sp.tile([128, 512], bf16, tag="wt", bufs=4)
                            nc.vector.tensor_mul(wt[:], pt[:], mk[:])
                            wts.append(wt)
                            if dump_mask and j == 0:
                                nc.sync.dma_start(dbg_mask[k, :, :], mk[:])
                            for m in range(4):
                                nc.tensor.matmul(
                                    aggs[m][:],
                                    lhsT=ht[:, 128 * m:128 * (m + 1)],
                                    rhs=wt[:], start=(k == 0),
                                    stop=(k == KT - 1))
                        wpair = sp.tile([128, 512], f32, tag="wpair", bufs=2)
                        nc.vector.tensor_add(wpair[:], wts[0][:], wts[1][:])
                        if kp == 0:
                            nc.vector.tensor_copy(wsum[:], wpair[:])
                        else:
                            nc.vector.tensor_add(wsum[:], wsum[:], wpair[:])
                    den = pp.tile([1, 512], f32, tag="den")
                    nc.tensor.matmul(den[:], lhsT=ones_c[:], rhs=wsum[:],
                                     start=True, stop=True)
                    inv = rp.tile([1, 512], f32, tag="inv")
                    nc.vector.reciprocal(inv[:], den[:])
                    invp = pp.tile([128, 512], f32, tag="invb")
                    nc.tensor.matmul(invp[:], lhsT=ones_r[:], rhs=inv[:],
                                     start=True, stop=True)
                    invs = rp.tile([128, 512], f32, tag="invs")
                    nc.vector.tensor_copy(invs[:], invp[:])
                    for m in range(4):
                        tmp = sp.tile([128, 512], f32, tag="tmp", bufs=2)
                        nc.vector.tensor_mul(tmp[:], aggs[m][:], invs[:])
                        write_out(j, m, tmp)

            # ---------- GAT layer 1 ----------
            compute_h(h_dram1, g1T, l0_dram, None)
            wsd1 = compute_wsd(g1, a1, "wsd1")
            ssrc1c = compute_ssrc_full(wsd1, l0_dram, None, "s1c")
            sdb1 = compute_sdst_b(wsd1, l0o, "sdb")
            if DEBUG:
                nc.sync.dma_start(dbg_h1[:], h_dram1[:])
                nc.sync.dma_start(dbg_ssrc[:], ssrc1c[:])
                nc.sync.dma_start(dbg_sdb[:], sdb1[:])
            l1own = op_.tile([128, 4, NO], bf16, tag="own")

            def write_l1(j, m, tmp):
                sl = l1own[:, m, 512 * j:512 * (j + 1)]
                nc.vector.tensor_scalar_add(sl, tmp[:], bg1f[:, m:m + 1])
                nc.sync.dma_start(
                    l1own_b[m, :, 512 * j:512 * (j + 1)], sl)

            gat_loop(h_dram1, ssrc1c, sdb1, write_l1, dump_mask=DEBUG)

            # ---------- AllGather l1 ----------
            nc.gpsimd.collective_compute(
                "AllGather", mybir.AluOpType.bypass,
                replica_groups=RG, ins=[l1own_b.opt()], outs=[l1g.opt()])

            # ---------- own-slice h2 + AllGather (in flight during attn) ----
            for tp in range(4):
                st2 = sp.tile([128, 2, 512], bf16, tag="stg2")
                for ti in range(2):
                    t = 2 * tp + ti
                    ps = ppmm.tile([128, 512], f32, tag="mm")
                    for k2 in range(4):
                        nc.tensor.matmul(
                            ps[:], lhsT=l1own[:, k2, 128 * t:128 * (t + 1)],
                            rhs=g2T[:, k2, :], start=(k2 == 0), stop=(k2 == 3))
                    nc.vector.tensor_copy(st2[:, ti, :], ps[:])
                nc.sync.dma_start(
                    h2o_b[2 * tp:2 * tp + 2, :, :].rearrange("a p c -> p a c"),
                    st2[:])
            nc.gpsimd.collective_compute(
                "AllGather", mybir.AluOpType.bypass,
                replica_groups=RG, ins=[h2o_b.opt()], outs=[h2g.opt()])

            if DEBUG:
                nc.sync.dma_start(dbg_l1[:], l1own_b[:])
                nc.sync.dma_start(dbg_l1g[:], l1g[:])

            # ---------- attention ----------
            at16 = rp.tile([128, 4, 512], bf16, tag="at16")
            for qh in range(2):
                avs = [pp.tile([128, 512], f32, tag=f"agg{m}",
                               name=f"av{m}") for m in range(4)]
                esum = rp.tile([128, 512], f32, tag="wsum")
                for kkp in range(KT // 2):
                    kk0 = 2 * kkp
                    cr = kk0 // 8
                    dl = kk0 % 8
                    ktile = lp.tile([128, 4, 256], bf16, tag="kst")
                    nc.sync.dma_start(
                        ktile[:], kg[cr, :, :, 128 * dl:128 * (dl + 2)]
                        .rearrange("a p c -> p a c"))
                    vtile = lp.tile([128, 2, 512], bf16, tag="vst")
                    nc.sync.dma_start(
                        vtile[:], vg[cr, dl:dl + 2, :, :]
                        .rearrange("a p c -> p a c"))
                    ess = []
                    for ki in range(2):
                        kk = kk0 + ki
                        pscr = ppmm.tile([128, 512], f32, tag="mm")
                        for k2 in range(4):
                            nc.tensor.matmul(
                                pscr[:],
                                lhsT=ktile[:, k2, 128 * ki:128 * (ki + 1)],
                                rhs=q16[:, k2, 512 * qh:512 * (qh + 1)],
                                start=(k2 == 0), stop=(k2 == 3))
                        es = sp.tile([128, 512], bf16, tag="es")
                        nc.scalar.activation(es[:], pscr[:], AF.Exp, scale=SCL)
                        ess.append(es)
                        for m in range(4):
                            nc.tensor.matmul(
                                avs[m][:],
                                lhsT=vtile[:, ki, 128 * m:128 * (m + 1)],
                                rhs=es[:], start=(kk == 0),
                                stop=(kk == KT - 1))
                    wpair = sp.tile([128, 512], f32, tag="wpair", bufs=2)
                    nc.vector.tensor_add(wpair[:], ess[0][:], ess[1][:])
                    if kkp == 0:
                        nc.vector.tensor_copy(esum[:], wpair[:])
                    else:
                        nc.vector.tensor_add(esum[:], esum[:], wpair[:])
                avden = pp.tile([1, 512], f32, tag="den")
                nc.tensor.matmul(avden[:], lhsT=ones_c[:], rhs=esum[:],
                                 start=True, stop=True)
                inv = rp.tile([1, 512], f32, tag="inv")
                nc.vector.reciprocal(inv[:], avden[:])
                invp = pp.tile([128, 512], f32, tag="invb")
                nc.tensor.matmul(invp[:], lhsT=ones_r[:], rhs=inv[:],
                                 start=True, stop=True)
                invs = rp.tile([128, 512], f32, tag="invs")
                nc.vector.tensor_copy(invs[:], invp[:])
                for m in range(4):
                    nc.vector.tensor_mul(at16[:, m, :], avs[m][:], invs[:])
                # output projection for this q-half
                for m in range(4):
                    ps = ppmm.tile([128, 512], f32, tag="mm")
                    for k2 in range(4):
                        nc.tensor.matmul(
                            ps[:], lhsT=oT[:, k2, 128 * m:128 * (m + 1)],
                            rhs=at16[:, k2, :], start=(k2 == 0), stop=(k2 == 3))
                    stf = sp.tile([128, 512], f32, tag="stgf", bufs=2)
                    nc.vector.tensor_scalar_add(stf[:], ps[:], bof2[:, m:m + 1])
                    nc.sync.dma_start(
                        out_g[m, :, 512 * qh:512 * (qh + 1)], stf[:])

            # ---------- GAT layer 2 ----------
            wsd2 = compute_wsd(g2, a2, "wsd2")
            ssrc2c = compute_ssrc_full(wsd2, None, l1g, "s2c")
            sdb2 = compute_sdst_b(wsd2, l1own, "sdb")

            def write_l2(j, m, tmp):
                stf = sp.tile([128, 512], f32, tag="stgf", bufs=2)
                nc.vector.tensor_scalar_add(stf[:], tmp[:], bg2f[:, m:m + 1])
                nc.sync.dma_start(
                    out_l[m, :, 512 * j:512 * (j + 1)], stf[:])

            gat_loop(None, ssrc2c, sdb2, write_l2, h_g=h2g)

    nc.finalize()
    return nc


def _prep_tables(src, dst):
    """Pack per-core edge tables for local_scatter mask construction."""
    per_core = []
    Wmax = 0
    for c in range(N_CORES):
        lo, hi = c * NO, (c + 1) * NO
        sel = (dst >= lo) & (dst < hi)
        s = src[sel].astype(np.int64)
        dl = (dst[sel] - lo).astype(np.int64)
        key = s * NO + dl
        uniq, counts = np.unique(key, return_counts=True)
        s_u = uniq // NO
        dl_u = uniq % NO
        k = s_u // 128
        p = s_u % 128
        col = dl_u
        bucket = k * 128 + p
        order = np.argsort(bucket, kind="stable")
        bucket = bucket[order]
        col = col[order]
        counts = counts[order]
        # slot index within each bucket
        bstart = np.r_[0, np.flatnonzero(np.diff(bucket)) + 1]
        sizes = np.diff(np.r_[bstart, bucket.size])
        slot = np.arange(bucket.size) - np.repeat(bstart, sizes)
        Wmax = max(Wmax, int(sizes.max()) if sizes.size else 0)
        per_core.append((bucket, col, counts, slot))
    W = max(2, (Wmax + 1) // 2 * 2)
    idx_tables, cnt_tables = [], []
    import ml_dtypes
    for bucket, col, counts, slot in per_core:
        sc_idx = np.full((128, KT * W), -1, np.int16)
        sc_cnt = np.zeros((128, KT * W), ml_dtypes.bfloat16)
        kj = bucket // 128
        p = bucket % 128
        flat = kj * W + slot
        sc_idx[p, flat] = col.astype(np.int16)
        sc_cnt[p, flat] = counts.astype(np.float32)
        idx_tables.append(sc_idx)
        cnt_tables.append(sc_cnt)
    return W, idx_tables, cnt_tables


def kernel(**inputs):
    global LAST_EXEC_NS
    from concourse.bass_utils import run_bass_kernel_spmd

    f = lambda name: np.ascontiguousarray(np.asarray(inputs[name], np.float32))
    x_A, x_B = f("x_A"), f("x_B")
    eAB = np.asarray(inputs["edge_AB"]).astype(np.int64)
    eBA = np.asarray(inputs["edge_BA"]).astype(np.int64)

    src = np.concatenate([eAB[0], eBA[0] + NA, np.arange(N, dtype=np.int64)])
    dst = np.concatenate([eAB[1] + NA, eBA[1], np.arange(N, dtype=np.int64)])
    W, idx_tables, cnt_tables = _prep_tables(src, dst)

    if W not in _CACHE:
        _CACHE[W] = _build(W)
    nc = _CACHE[W]

    xT = np.ascontiguousarray(np.concatenate([x_A, x_B], 0).T)
    col = lambda name: f(name).reshape(-1, 1)
    WqkvT = f("Wqkv").T  # [H, 3H]
    shared = {
        "xT": xT,
        "WinA_T": np.ascontiguousarray(f("W_inA").T),
        "WinB_T": np.ascontiguousarray(f("W_inB").T),
        "Win2A_T": np.ascontiguousarray(f("W_in2A").T),
        "Win2B_T": np.ascontiguousarray(f("W_in2B").T),
        "binA": col("b_inA"), "binB": col("b_inB"),
        "bin2A": col("b_in2A"), "bin2B": col("b_in2B"),
        "Wg1": f("Wg1"), "Wg1_T": np.ascontiguousarray(f("Wg1").T),
        "A1": np.ascontiguousarray(
            np.stack([f("a_src1"), f("a_dst1")], 1)),
        "bg1": col("bg1"),
        "Wg2": f("Wg2"), "Wg2_T": np.ascontiguousarray(f("Wg2").T),
        "A2": np.ascontiguousarray(
            np.stack([f("a_src2"), f("a_dst2")], 1)),
        "bg2": col("bg2"),
        "WqT": np.ascontiguousarray(WqkvT[:, 0:H]),
        "WkT": np.ascontiguousarray(WqkvT[:, H:2 * H]),
        "WvT": np.ascontiguousarray(WqkvT[:, 2 * H:3 * H]),
        "bq": col("bqkv")[0:H], "bk": col("bqkv")[H:2 * H],
        "bv": col("bqkv")[2 * H:3 * H],
        "WoT": np.ascontiguousarray(f("Wo").T), "bo": col("bo"),
    }
    in_maps = []
    for c in range(N_CORES):
        m = dict(shared)
        m["xoT"] = np.ascontiguousarray(xT[:, c * NO:(c + 1) * NO])
        if c < N_CORES // 2:
            m["win_o"] = shared["WinA_T"]; m["bin_o"] = shared["binA"]
            m["win2_o"] = shared["Win2A_T"]; m["bin2_o"] = shared["bin2A"]
        else:
            m["win_o"] = shared["WinB_T"]; m["bin_o"] = shared["binB"]
            m["win2_o"] = shared["Win2B_T"]; m["bin2_o"] = shared["bin2B"]
        m["sc_idx"] = idx_tables[c]
        m["sc_cnt"] = cnt_tables[c]
        in_maps.append(m)

    if TRACE:
        _install_trace_hook()
    res = run_bass_kernel_spmd(nc, in_maps, list(range(N_CORES)),
                               trace=bool(TRACE))
    LAST_EXEC_NS = res.exec_time_ns
    global _LAST_RES
    _LAST_RES = res

    l_full = np.empty((N, H), np.float32)
    g_full = np.empty((N, H), np.float32)
    for c in range(N_CORES):
        r = res.results[c]
        l_full[c * NO:(c + 1) * NO] = r["out_l"].reshape(H, NO).T
        g_full[c * NO:(c + 1) * NO] = r["out_g"].reshape(H, NO).T
    z_A = np.concatenate([l_full[:NA], g_full[:NA]], 1)
    z_B = np.concatenate([l_full[NA:], g_full[NA:]], 1)
    return (z_A, z_B)
